# revision 16
# baseline (speedup 1.0000x reference)
"""Causal self-attention, tensor-parallel over heads across 8 TRN2 NeuronCores.

B=2, T=2048, C=1024, H=16 heads, D=64. Each core owns 2 heads (128 cols of C)
for both batches, computes QKV projections + causal attention + the softmax
normalizer (via a ones-column appended to v), then two per-head AllToAlls
convert the column-sharded attention output y^T into row shards so each core
computes a disjoint 512-row slice of the final Wo projection; the first A2A
and half the Wo contraction overlap the second head's attention compute.
bf16 matmuls, fp32 PSUM accumulation.
"""

import sys

sys.path.insert(0, "/opt/trn_rl_repo")

import numpy as np
import ml_dtypes

import concourse.bass as bass
import concourse.bacc as bacc
import concourse.mybir as mybir
from concourse.tile import TileContext
from concourse.masks import make_identity
from concourse import bass_utils

BF16 = mybir.dt.bfloat16
F32 = mybir.dt.float32
NPBF16 = ml_dtypes.bfloat16

B, T, C, H, D = 2, 2048, 1024, 16, 64
NCORES = 8
HL = H // NCORES          # heads per core = 2
COLS = HL * D             # 128 head-cols per core
KT = C // 128             # 8 contraction k-tiles
NCH = T // 512            # 4 query chunks of 512 per batch
NT = T // 128             # 16 key tiles of 128 per batch
VW = D + 1                # 65: v columns + ones column
ROWS_PER_CORE = B * T // NCORES  # 512 output rows per core

MASK_NEG = -60000.0
SCALE = 1.0 / np.sqrt(np.float32(D))
STAGE = "AB2C"
MASK_PE = True
MEMSET_GPS = True

Exp = mybir.ActivationFunctionType.Exp
Copy = mybir.ActivationFunctionType.Copy


def build_nc():
    nc = bacc.Bacc(
        "TRN2",
        target_bir_lowering=False,
        debug=False,
        enable_asserts=False,
        num_devices=NCORES,
    )
    xT = nc.dram_tensor("xT", [C, B * T], BF16, kind="ExternalInput")
    wq = nc.dram_tensor("wq", [C, COLS], BF16, kind="ExternalInput")
    wk = nc.dram_tensor("wk", [C, COLS], BF16, kind="ExternalInput")
    wv = nc.dram_tensor("wv", [C, COLS], BF16, kind="ExternalInput")
    # wo rows pre-permuted on host: h-major [h, core, 64]
    wo = nc.dram_tensor("wo", [C, C], BF16, kind="ExternalInput")
    bqk = nc.dram_tensor("bqk", [COLS, 2], F32, kind="ExternalInput")
    bv = nc.dram_tensor("bv", [1, COLS], BF16, kind="ExternalInput")
    bo = nc.dram_tensor("bo", [1, C], BF16, kind="ExternalInput")
    mtri = nc.dram_tensor("mtri", [128, 128], BF16, kind="ExternalInput")
    send = [
        nc.dram_tensor(f"a2a_send{h}", [NCORES * D, ROWS_PER_CORE], BF16)
        for h in range(HL)
    ]
    recv = [
        nc.dram_tensor(f"a2a_recv{h}", [NCORES * D, ROWS_PER_CORE], BF16)
        for h in range(HL)
    ]
    out = nc.dram_tensor("out", [ROWS_PER_CORE, C], F32, kind="ExternalOutput")

    add = mybir.AluOpType.add
    mult = mybir.AluOpType.mult

    def load_w3(dst, src, width):
        """One DMA: dst [128, KT*width] <- src [C, width] with k-tile blocks."""
        nc.sync.dma_start(
            out=dst.rearrange("p (k w) -> p k w", k=KT),
            in_=src.rearrange("(k p) w -> p k w", k=KT),
        )

    def stage_a(x_sb, psA, cs):
        for b in range(B):
            for proj, wsb, osb, bcol in (
                ("q", cs["wq"], cs["qT"], 0),
                ("k", cs["wk"], cs["kT"], 1),
            ):
                for n in range(NCH):
                    ps = psA.tile([128, 512], F32, tag="psA", name="psA")
                    col = b * T + n * 512
                    for k in range(KT):
                        nc.tensor.matmul(
                            ps[:],
                            wsb[:, k * COLS : (k + 1) * COLS],
                            x_sb[k][:, col : col + 512],
                            start=(k == 0),
                            stop=(k == KT - 1),
                        )
                    nc.vector.tensor_scalar(
                        osb[:, col : col + 512],
                        ps[:],
                        cs["bqk"][:, bcol : bcol + 1],
                        None,
                        add,
                    )
            # v: natural layout [T-tile, cols]
            for m in range(NT):
                ps = psA.tile([128, COLS], F32, tag="psAv", name="psAv")
                col = b * T + m * 128
                for k in range(KT):
                    nc.tensor.matmul(
                        ps[:],
                        x_sb[k][:, col : col + 128],
                        cs["wv"][:, k * COLS : (k + 1) * COLS],
                        start=(k == 0),
                        stop=False,
                    )
                nc.tensor.matmul(
                    ps[:],
                    cs["ones"][0:1, :],
                    cs["bv"][0:1, :],
                    start=False,
                    stop=True,
                )
                # v_sb [128, B*NT*2, VW]; ones column (idx D) preset by memset
                vi = (b * NT + m) * HL
                nc.vector.tensor_copy(
                    out=cs["v"][:, vi : vi + HL, 0:D],
                    in_=ps[:, :].rearrange("p (h d) -> p h d", h=HL),
                )

    def stage_b_head(h, ptp, psS, psY, psB, nrm, cs):
        for b in range(B):
            for n in range(NCH):
                qcol = b * T + n * 512
                hp = slice(h * D, (h + 1) * D)
                pts = []
                for m in range(4 * n + 4):
                    pt = ptp.tile([128, 512], BF16, tag="pt", name="pt")
                    ps = psS.tile([128, 512], F32, tag="psS", name="psS")
                    kcol = b * T + m * 128
                    if m < 4 * n:
                        nc.tensor.matmul(
                            ps[:],
                            cs["kT"][hp, kcol : kcol + 128],
                            cs["qT"][hp, qcol : qcol + 512],
                            start=True,
                            stop=True,
                        )
                        nc.scalar.activation(pt[:], ps[:], Exp, scale=float(SCALE))
                    else:
                        j = m - 4 * n
                        for sq in range(j, 4):
                            nc.tensor.matmul(
                                ps[:, sq * 128 : (sq + 1) * 128],
                                cs["kT"][hp, kcol : kcol + 128],
                                cs["qT"][
                                    hp, qcol + sq * 128 : qcol + (sq + 1) * 128
                                ],
                                start=True,
                                stop=(sq != j or not MASK_PE),
                            )
                            if sq == j and MASK_PE:
                                # causal mask of the diagonal block via PE:
                                # ps[:, j] += I.T @ mtri (adjacent in the
                                # accumulation group so no other start=True
                                # clears the region's has_written bits)
                                nc.tensor.matmul(
                                    ps[:, j * 128 : (j + 1) * 128],
                                    cs["ident"][:],
                                    cs["mtri"][:],
                                    start=False,
                                    stop=True,
                                )
                        if not MASK_PE:
                            nc.vector.tensor_tensor(
                                ps[:, j * 128 : (j + 1) * 128],
                                ps[:, j * 128 : (j + 1) * 128],
                                cs["mtri"][:],
                                add,
                            )
                        if j > 0:
                            (nc.gpsimd if MEMSET_GPS else nc.vector).memset(
                                pt[:, 0 : j * 128], 0.0
                            )
                        nc.scalar.activation(
                            pt[:, j * 128 :],
                            ps[:, j * 128 :],
                            Exp,
                            scale=float(SCALE),
                        )
                    pts.append(pt)

                # y^T (+ sum row) = v_aug^T @ P^T
                py = psY.tile([VW, 512], F32, tag="psY", name="psY")
                last = 4 * n + 3
                for m in range(4 * n + 4):
                    vi = (b * NT + m) * HL + h
                    nc.tensor.matmul(
                        py[:],
                        cs["v"][:, vi : vi + 1, :],
                        pts[m][:],
                        start=(m == 0),
                        stop=(m == last),
                    )
                inv = nrm.tile([1, 512], BF16, tag="inv", name="inv")
                with nc.allow_low_precision(reason="bf16 softmax norm, tol 2e-2"):
                    nc.vector.reciprocal(inv[:], py[D : D + 1, :])
                pb = psB.tile([D, 512], F32, tag="psB", name="psB")
                nc.tensor.matmul(
                    pb[:], cs["ones"][0:1, 0:D], inv[:], start=True, stop=True
                )
                binv = nrm.tile([D, 512], BF16, tag="binv", name="binv")
                nc.scalar.activation(binv[:], pb[:], Copy)
                yn = nrm.tile([D, 512], BF16, tag="yn", name="yn")
                nc.vector.tensor_tensor(yn[:], py[0:D, :], binv[:], mult)
                shard = 4 * b + n
                nc.sync.dma_start(
                    out=send[h][shard * D : (shard + 1) * D, :], in_=yn[:]
                )

    def a2a(h):
        nc.gpsimd.collective_compute(
            "AllToAll",
            mybir.AluOpType.bypass,
            replica_groups=[list(range(NCORES))],
            ins=[send[h][:]],
            outs=[recv[h][:]],
        )

    def stage_c_half(h, cp, psC, cs, acc):
        """Wo contraction over the 4 k-tiles supplied by recv[h]."""
        y_sb = cp.tile([128, 4 * 512], BF16, tag=f"ysb{h}", name=f"ysb{h}")
        nc.sync.dma_start(
            out=y_sb.rearrange("p (k r) -> p k r", k=4),
            in_=recv[h].rearrange("(k p) r -> p k r", k=4),
        )
        for r in range(ROWS_PER_CORE // 128):
            for o in range(C // 512):
                pc = psC.tile([128, 512], F32, tag="psC", name="psC")
                for k in range(4):
                    kk = h * 4 + k
                    nc.tensor.matmul(
                        pc[:],
                        y_sb[:, k * 512 + r * 128 : k * 512 + r * 128 + 128],
                        cs["wo"][:, kk * C + o * 512 : kk * C + (o + 1) * 512],
                        start=(k == 0),
                        stop=(h == 1 and k == 3),
                    )
                if h == 0:
                    nc.tensor.matmul(
                        pc[:],
                        cs["ones"][0:1, :],
                        cs["bo"][0:1, o * 512 : (o + 1) * 512],
                        start=False,
                        stop=True,
                    )
                    a = cp.tile([128, 512], F32, tag="acc", name="acc", bufs=8)
                    nc.vector.tensor_copy(out=a[:], in_=pc[:])
                    acc[(r, o)] = a
                else:
                    osb = cp.tile([128, 512], F32, tag="osb", name="osb", bufs=3)
                    nc.vector.tensor_tensor(osb[:], pc[:], acc[(r, o)][:], add)
                    nc.sync.dma_start(
                        out=out[r * 128 : (r + 1) * 128, o * 512 : (o + 1) * 512],
                        in_=osb[:],
                    )

    with TileContext(nc) as tc:
        with tc.tile_pool(name="persist", bufs=1) as pp:
            cs = {}
            # weights first (needed with first x tiles)
            cs["wq"] = pp.tile([128, KT * COLS], BF16, tag="wq", name="wq")
            cs["wk"] = pp.tile([128, KT * COLS], BF16, tag="wk", name="wk")
            cs["wv"] = pp.tile([128, KT * COLS], BF16, tag="wv", name="wv")
            load_w3(cs["wq"], wq[:], COLS)
            load_w3(cs["wk"], wk[:], COLS)
            load_w3(cs["wv"], wv[:], COLS)

            with tc.tile_pool(name="xp", bufs=1) as xp:
                x_sb = []
                for k in range(KT):
                    xt = xp.tile([128, B * T], BF16, tag=f"x{k}", name=f"x{k}")
                    nc.sync.dma_start(out=xt[:], in_=xT[k * 128 : (k + 1) * 128, :])
                    x_sb.append(xt)

                # small constants (issued on scalar queue to not delay x)
                cs["ones"] = pp.tile([1, 128], BF16, tag="ones", name="ones")
                nc.vector.memset(cs["ones"][:], 1.0)
                cs["bqk"] = pp.tile([COLS, 2], F32, tag="bqk", name="bqk")
                nc.scalar.dma_start(out=cs["bqk"][:], in_=bqk[:])
                cs["bv"] = pp.tile([1, COLS], BF16, tag="bv", name="bv")
                nc.scalar.dma_start(out=cs["bv"][:], in_=bv[:])
                cs["bo"] = pp.tile([1, C], BF16, tag="bo", name="bo")
                nc.scalar.dma_start(out=cs["bo"][:], in_=bo[:])
                cs["mtri"] = pp.tile([128, 128], BF16, tag="mtri", name="mtri")
                nc.scalar.dma_start(out=cs["mtri"][:], in_=mtri[:])
                cs["ident"] = pp.tile([128, 128], BF16, tag="ident", name="ident")
                make_identity(nc, cs["ident"][:])

                cs["qT"] = pp.tile([128, B * T], BF16, tag="qT", name="qT")
                cs["kT"] = pp.tile([128, B * T], BF16, tag="kT", name="kT")
                cs["v"] = pp.tile([128, B * NT * HL, VW], BF16, tag="v", name="v")
                nc.gpsimd.memset(cs["v"][:], 1.0)  # presets the ones columns

                # wo loaded last (not needed until stage C)
                cs["wo"] = pp.tile([128, KT * C], BF16, tag="wo", name="wo")
                nc.scalar.dma_start(
                    out=cs["wo"].rearrange("p (k w) -> p k w", k=KT),
                    in_=wo.rearrange("(k p) w -> p k w", k=KT),
                )

                with tc.tile_pool(name="psA", bufs=4, space="PSUM") as psA:
                    if "A" in STAGE:
                        stage_a(x_sb, psA, cs)

            acc = {}
            if "B" in STAGE:
                with tc.tile_pool(name="pt", bufs=40) as ptp, tc.tile_pool(
                    name="psS", bufs=3, space="PSUM"
                ) as psS, tc.tile_pool(
                    name="psY", bufs=2, space="PSUM"
                ) as psY, tc.tile_pool(
                    name="psB", bufs=1, space="PSUM"
                ) as psB, tc.tile_pool(
                    name="nrm", bufs=4
                ) as nrm, tc.tile_pool(
                    name="cp", bufs=1
                ) as cp, tc.tile_pool(
                    name="psC", bufs=2, space="PSUM"
                ) as psC:
                    stage_b_head(0, ptp, psS, psY, psB, nrm, cs)
                    if "2" in STAGE:
                        a2a(0)
                    stage_b_head(1, ptp, psS, psY, psB, nrm, cs)
                    if "2" in STAGE:
                        if "C" in STAGE:
                            stage_c_half(0, cp, psC, cs, acc)
                        a2a(1)
                        if "C" in STAGE:
                            stage_c_half(1, cp, psC, cs, acc)
    nc.compile()
    return nc


def make_in_maps(x, mask, Wq, bq, Wk, bk, Wv, bv, Wo, bo):
    xT = np.ascontiguousarray(
        x.astype(np.float32).transpose(2, 0, 1).reshape(C, B * T)
    ).astype(NPBF16)
    mtri = np.where(
        np.arange(128)[:, None] > np.arange(128)[None, :], MASK_NEG, 0.0
    ).astype(NPBF16)
    # Wo rows permuted h-major: new row order = [core0 h0 d0..63, core1 h0, ...,
    # core7 h0, core0 h1, ..., core7 h1]
    perm = np.concatenate(
        [
            np.arange(c * COLS + h * D, c * COLS + h * D + D)
            for h in range(HL)
            for c in range(NCORES)
        ]
    )
    wo_b = np.ascontiguousarray(Wo[perm]).astype(NPBF16)
    bo_b = bo.reshape(1, C).astype(NPBF16)
    in_maps = []
    for c in range(NCORES):
        cslice = slice(c * COLS, (c + 1) * COLS)
        in_maps.append(
            {
                "xT": xT,
                "wq": np.ascontiguousarray(Wq[:, cslice]).astype(NPBF16),
                "wk": np.ascontiguousarray(Wk[:, cslice]).astype(NPBF16),
                "wv": np.ascontiguousarray(Wv[:, cslice]).astype(NPBF16),
                "wo": wo_b,
                "bqk": np.stack([bq[cslice], bk[cslice]], axis=1).astype(
                    np.float32
                ),
                "bv": bv[cslice].reshape(1, COLS).astype(NPBF16),
                "bo": bo_b,
                "mtri": mtri,
            }
        )
    return in_maps


_CACHED_NC = None


def run(inputs, trace=False, **kw):
    global _CACHED_NC
    if _CACHED_NC is None:
        _CACHED_NC = build_nc()
    in_maps = make_in_maps(**inputs)
    res = bass_utils.run_bass_kernel_spmd(
        _CACHED_NC, in_maps, core_ids=list(range(NCORES)), trace=trace, **kw
    )
    outs = [np.asarray(res.results[c]["out"]) for c in range(NCORES)]
    full = np.concatenate(outs, axis=0).reshape(B, T, C).astype(np.float32)
    return full, res


def kernel(**inputs):
    full, _ = run(inputs, trace=False)
    return full
